# revision 12
# baseline (speedup 1.0000x reference)
"""Bass/Tile TRN2 kernel: adaptive min 2D pooling (8x8 grid) of [B,512,512] f32.

Full input [128, 512, 512] f32 -> output [128, 64] f32.
Data parallel over 8 NeuronCores: 16 matrices per core.

Per-core algorithm (x_local [16, 512, 512] -> y_local [16, 64]):
  1. For each matrix m: DMA [512,512] into SBUF as [128, 2048] with
     partition p = row within a 128-row quarter, free = (q, c):
     row = q*128 + p. Each partition line is 4 chunks of 2KB contiguous
     HBM -> near line-rate DMA.
  2. DVE reduce_min over the innermost 64-column groups:
     [128, (q gc c=64)] -> acc[:, m*32 + (q*8+gc)]  (min over c).
     acc is [128, 512] with free = (m, q, gc), partition = row-in-quarter.
  3. Cross-partition min (over the 128 rows-in-quarter = 2 bands x 64 rows)
     cannot run on DVE -> PE-transpose each 128-col block of acc into PSUM
     (exact data movement), then DVE reduce_min over the row halves:
     accT_k [128=(m' q gc), (b=2, r=64)] -> res[:, 2k+b].
  4. One small DMA scatters res [128, 8] into y [16, 64].
"""

import threading

import numpy as np

B, N, M = 128, 512, 512
GRID = 8
NCORES = 8
BL = B // NCORES  # 16 matrices per core

_lock = threading.Lock()
_cache: dict = {}


def _build(n_iters: int = 1, bufs: int = 4):
    import concourse.bacc as bacc
    import concourse.mybir as mybir
    import concourse.tile as tile

    f32 = mybir.dt.float32

    nc = bacc.Bacc("TRN2", target_bir_lowering=False, debug=False)
    x = nc.dram_tensor("x", [BL, N, M], f32, kind="ExternalInput").ap()
    y = nc.dram_tensor("y", [BL, GRID * GRID], f32, kind="ExternalOutput").ap()

    with tile.TileContext(nc) as tc:
        with (
            tc.tile_pool(name="inp", bufs=bufs) as inp,
            tc.tile_pool(name="accp", bufs=2) as accp,
            tc.tile_pool(name="resp", bufs=2) as resp,
            tc.tile_pool(name="idp", bufs=1) as idp,
            tc.tile_pool(name="ps", bufs=4, space="PSUM") as ps,
        ):
            # identity matrix for the PE transpose
            ones = idp.tile([128, 128], f32)
            ident = idp.tile([128, 128], f32)
            nc.gpsimd.memset(ones[:], 1.0)
            nc.gpsimd.affine_select(
                ident[:],
                ones[:],
                pattern=[[-1, 128]],
                compare_op=mybir.AluOpType.is_equal,
                fill=0.0,
                base=0,
                channel_multiplier=1,
            )

            for _ in range(n_iters):  # n_iters>1 only for benchmarking
                acc = accp.tile([128, 512], f32)
                res = resp.tile([128, 8], f32)

                # stage 1: per-matrix load + min over column groups.
                # Alternate the two HWDGE rings (SP / ACT) — a single ring
                # serializes with ~1us bubbles between DMAs (300 GB/s); the
                # two rings together reach the HBM roofline (~373 GB/s).
                # The last matrix is split into 4 quarter DMAs/reduces so the
                # kernel tail only waits on a 256KB transfer + small reduce
                # (-10us measured). Interleaving stage 2 into this stream was
                # measured SLOWER on HW (+12-30us; PE/PSUM traffic and scatter
                # DMAs stall the FIFO DMA rings mid-stream), so stage 2 stays
                # at the end.
                for m in range(BL):
                    t = inp.tile([128, 4 * M], f32)
                    if m == BL - 1:
                        for q in range(4):
                            eng = nc.sync if q % 2 == 0 else nc.scalar
                            eng.dma_start(
                                t[:, q * M : (q + 1) * M],
                                x[m, q * 128 : (q + 1) * 128, :],
                            )
                            nc.vector.tensor_reduce(
                                acc[:, m * 32 + q * 8 : m * 32 + (q + 1) * 8],
                                t[:, q * M : (q + 1) * M].rearrange(
                                    "p (g c) -> p g c", c=M // GRID
                                ),
                                axis=mybir.AxisListType.X,
                                op=mybir.AluOpType.min,
                            )
                    else:
                        eng = nc.sync if m % 2 == 0 else nc.scalar
                        eng.dma_start(
                            t[:].rearrange("p (q c) -> p q c", q=4),
                            x[m].rearrange("(q p) c -> p q c", p=128),
                        )
                        nc.vector.tensor_reduce(
                            acc[:, m * 32 : (m + 1) * 32],
                            t[:].rearrange("p (g c) -> p g c", c=M // GRID),
                            axis=mybir.AxisListType.X,
                            op=mybir.AluOpType.min,
                        )

                # stage 2: cross-partition min via PE transpose + free-dim
                # reduce over the row halves (bands).
                for k in range(4):
                    pt = ps.tile([128, 128], f32)
                    nc.tensor.transpose(
                        pt[:], acc[:, k * 128 : (k + 1) * 128], ident[:]
                    )
                    nc.vector.tensor_reduce(
                        res[:, k * 2 : (k + 1) * 2],
                        pt[:].rearrange("p (b r) -> p b r", b=2),
                        axis=mybir.AxisListType.X,
                        op=mybir.AluOpType.min,
                    )

                # res[(m' q gc), (k b)] -> y[4k+m', (2q+b)*8+gc]
                for k in range(4):
                    for b in range(2):
                        eng = nc.sync if (2 * k + b) % 2 == 0 else nc.scalar
                        eng.dma_start(
                            y[4 * k : 4 * (k + 1)].rearrange(
                                "mp (q b gc) -> mp q b gc", q=4, b=2
                            )[:, :, b, :],
                            res[:, k * 2 + b],
                        )

    nc.compile()
    return nc


def _get_nc():
    with _lock:
        if "nc" not in _cache:
            _cache["nc"] = _build()
        return _cache["nc"]


def kernel(sim_matrices: np.ndarray) -> np.ndarray:
    from concourse.bass_utils import run_bass_kernel_spmd

    nc = _get_nc()
    xs = np.ascontiguousarray(sim_matrices, dtype=np.float32)
    in_maps = [{"x": xs[i * BL : (i + 1) * BL]} for i in range(NCORES)]
    r = run_bass_kernel_spmd(nc, in_maps, list(range(NCORES)))
    return np.concatenate([r.results[i]["y"] for i in range(NCORES)], axis=0)


# revision 13
# speedup vs baseline: 1.4159x; 1.4159x over previous
"""Bass/Tile TRN2 kernel: adaptive min 2D pooling (8x8 grid) of [B,512,512] f32.

Full input [128, 512, 512] f32 -> output [128, 64] f32.
Data parallel over 8 NeuronCores: 16 matrices per core.

Per-core algorithm (x_local [16, 512, 512] -> y_local [16, 64]):
  1. For each matrix m: DMA [512,512] into SBUF as [128, 2048] with
     partition p = row within a 128-row quarter, free = (q, c):
     row = q*128 + p. Each partition line is 4 chunks of 2KB contiguous
     HBM -> near line-rate DMA.
  2. DVE reduce_min over the innermost 64-column groups:
     [128, (q gc c=64)] -> acc[:, m*32 + (q*8+gc)]  (min over c).
     acc is [128, 512] with free = (m, q, gc), partition = row-in-quarter.
  3. Cross-partition min (over the 128 rows-in-quarter = 2 bands x 64 rows)
     cannot run on DVE -> PE-transpose each 128-col block of acc into PSUM
     (exact data movement), then DVE reduce_min over the row halves:
     accT_k [128=(m' q gc), (b=2, r=64)] -> res[:, 2k+b].
  4. One small DMA scatters res [128, 8] into y [16, 64].
"""

import threading

import numpy as np

B, N, M = 128, 512, 512
GRID = 8
NCORES = 8
BL = B // NCORES  # 16 matrices per core

_lock = threading.Lock()
_cache: dict = {}


def _build(n_iters: int = 1, bufs: int = 6):
    import concourse.bacc as bacc
    import concourse.mybir as mybir
    import concourse.tile as tile

    f32 = mybir.dt.float32

    nc = bacc.Bacc("TRN2", target_bir_lowering=False, debug=False)
    x = nc.dram_tensor("x", [BL, N, M], f32, kind="ExternalInput").ap()
    y = nc.dram_tensor("y", [BL, GRID * GRID], f32, kind="ExternalOutput").ap()

    with tile.TileContext(nc) as tc:
        with (
            tc.tile_pool(name="inp", bufs=bufs) as inp,
            tc.tile_pool(name="accp", bufs=2) as accp,
            tc.tile_pool(name="resp", bufs=2) as resp,
            tc.tile_pool(name="idp", bufs=1) as idp,
            tc.tile_pool(name="ps", bufs=4, space="PSUM") as ps,
        ):
            # identity matrix for the PE transpose
            ones = idp.tile([128, 128], f32)
            ident = idp.tile([128, 128], f32)
            nc.gpsimd.memset(ones[:], 1.0)
            nc.gpsimd.affine_select(
                ident[:],
                ones[:],
                pattern=[[-1, 128]],
                compare_op=mybir.AluOpType.is_equal,
                fill=0.0,
                base=0,
                channel_multiplier=1,
            )

            for _ in range(n_iters):  # n_iters>1 only for benchmarking
                acc = accp.tile([128, 512], f32)
                res = resp.tile([128, 8], f32)

                # stage 1: per-matrix load + min over column groups.
                # Alternate the two HWDGE rings (SP / ACT) — a single ring
                # serializes with ~1us bubbles between DMAs (300 GB/s); the
                # two rings together reach the HBM roofline (~373 GB/s).
                # The last matrix is split into 4 quarter DMAs/reduces so the
                # kernel tail only waits on a 256KB transfer + small reduce
                # (-10us measured). Interleaving stage 2 into this stream was
                # measured SLOWER on HW (+12-30us; PE/PSUM traffic and scatter
                # DMAs stall the FIFO DMA rings mid-stream), so stage 2 stays
                # at the end.
                for m in range(BL):
                    t = inp.tile([128, 4 * M], f32)
                    if m == BL - 1:
                        for q in range(4):
                            eng = nc.sync if q % 2 == 0 else nc.scalar
                            eng.dma_start(
                                t[:, q * M : (q + 1) * M],
                                x[m, q * 128 : (q + 1) * 128, :],
                            )
                            nc.vector.tensor_reduce(
                                acc[:, m * 32 + q * 8 : m * 32 + (q + 1) * 8],
                                t[:, q * M : (q + 1) * M].rearrange(
                                    "p (g c) -> p g c", c=M // GRID
                                ),
                                axis=mybir.AxisListType.X,
                                op=mybir.AluOpType.min,
                            )
                    else:
                        eng = nc.sync if m % 2 == 0 else nc.scalar
                        eng.dma_start(
                            t[:].rearrange("p (q c) -> p q c", q=4),
                            x[m].rearrange("(q p) c -> p q c", p=128),
                        )
                        nc.vector.tensor_reduce(
                            acc[:, m * 32 : (m + 1) * 32],
                            t[:].rearrange("p (g c) -> p g c", c=M // GRID),
                            axis=mybir.AxisListType.X,
                            op=mybir.AluOpType.min,
                        )

                # stage 2: cross-partition min via PE transpose + free-dim
                # reduce over the row halves (bands).
                for k in range(4):
                    pt = ps.tile([128, 128], f32)
                    nc.tensor.transpose(
                        pt[:], acc[:, k * 128 : (k + 1) * 128], ident[:]
                    )
                    nc.vector.tensor_reduce(
                        res[:, k * 2 : (k + 1) * 2],
                        pt[:].rearrange("p (b r) -> p b r", b=2),
                        axis=mybir.AxisListType.X,
                        op=mybir.AluOpType.min,
                    )

                # res[(m' q gc), (k b)] -> y[4k+m', (2q+b)*8+gc]
                for k in range(4):
                    for b in range(2):
                        eng = nc.sync if (2 * k + b) % 2 == 0 else nc.scalar
                        eng.dma_start(
                            y[4 * k : 4 * (k + 1)].rearrange(
                                "mp (q b gc) -> mp q b gc", q=4, b=2
                            )[:, :, b, :],
                            res[:, k * 2 + b],
                        )

    nc.compile()
    return nc


def _get_nc():
    with _lock:
        if "nc" not in _cache:
            _cache["nc"] = _build()
        return _cache["nc"]


def kernel(sim_matrices: np.ndarray) -> np.ndarray:
    from concourse.bass_utils import run_bass_kernel_spmd

    nc = _get_nc()
    xs = np.ascontiguousarray(sim_matrices, dtype=np.float32)
    in_maps = [{"x": xs[i * BL : (i + 1) * BL]} for i in range(NCORES)]
    r = run_bass_kernel_spmd(nc, in_maps, list(range(NCORES)))
    return np.concatenate([r.results[i]["y"] for i in range(NCORES)], axis=0)
